# revision 79
# baseline (speedup 1.0000x reference)
"""Multi-head attention TRN2 kernel (v3: transposed-AV + pipelined pairs).

Problem: B=8, S=1024, D=768, H=16, Hd=48 MHA (dense_transformer).
Sharding: pure data parallel — one batch element per NeuronCore (8 cores).

Per-core device kernel:
  xT  [D, S]   host-pre-transposed, plain chunk DMAs
  qT  [D, S]   = (Wq/sqrt(Hd))^T @ xT + bq/sqrt(Hd)   (head-pair col layout)
  kT  [D, S]   = Wk^T @ xT + bk                        (head-pair col layout)
  v   [S, D]   = x @ Wv      stored per-head as [ones | 48 dims] 49-col blocks
  per pair p = heads (2p, 2p+1), per s_k chunk m:
    scoresT[S_k, S_q] = kT_h^T-contract qT_h  (K=48, two heads packed per PE
                        pass via row tile_position); scA holds the s_q-n0
                        half of both heads, scB the n1 half, so exp(n0) can
                        fire before the n1 projections even exist
    U = exp(scoresT)   (ACT engine; no max subtraction; scores ~ N(0,1))
  AV in the TRANSPOSED orientation: for s_q chunk c, head h, accumulate
    av[s_q, 0:49] += U_h[s_k chunk m, 128c:+128]^T @ [1 | v_h][s_k chunk m]
  so each AV matmul is M=128 (s_q), K=128 (s_k), N=49 — the cost model
  charges N only: AV is 50176 PE rows instead of v1's 131072. Slot 0 is the
  softmax denominator (ones column) -> normalization is a per-partition
  tensor_scalar multiply, no partition broadcast.
  AV(p-1) chunks execute during pair p's m-steps 4..7 (software pipeline one
  pair back) so exp(p-1,m) -> AV(p-1,m) handoff and the av-psum WAR against
  normalize(p-2) both have a full pair of slack.
  ao[c] [S_q chunk, D] fp16 (natural dim order) -> DMA-transpose (sync
  queue HWDGE, SBUF->SBUF [128,128] blocks, emitted as soon as every pair
  covering dim block j has normalized) -> aoT [D, S] dense
  yT [D, S] = Wo^T @ aoT + (bo + bv @ Wo)   (dense 6x6 contraction)

Layout invariants driven by hardware rules:
- engine SBUF/PSUM access patterns must start at partition 0/32/64/96, so
  qT/kT keep the 2-heads-per-128-partition pair layout (rows 0:48, 64:112)
- a matmul start=True marks pending-zero for its WHOLE psum bank (2KB zero
  region); the av tile runs one multi-slot accumulation group per bank
  (start=True on the bank's first matmul, each slot's first write stores,
  later writes accumulate, stop=True on the bank's last matmul)
- psum budget exactly 8 banks: scA(2) + scB(2) + av(2) + mm 2x[128,512](2)
- projection groups are deadline-scheduled into the attention m-steps with
  an even-spread floor so the PE has fill-in work under exp for ALL pairs
- HWDGE DMA transposes go on the otherwise-idle sync queue; bulk DMAs on
  gpsimd SWDGE, ordered (wq_k, wk_k, x_k) interleaved so the first q/k
  projection group is DMA-fed chunk by chunk from ~1.5us
"""

import math

import numpy as np

B, S, D = 8, 1024, 768
H, HD = 16, 48
PAIRS = H // 2
N_CORES = 8

_CACHE = {}
LAST_RESULTS = None
LAST_IN_MAPS = None


def _build_nc(reps=1, skip_attention=False):
    import concourse.bass as bass
    import concourse.mybir as mybir
    import concourse.tile as tile
    from concourse import bacc
    from concourse.bass import ts

    f16 = mybir.dt.float16
    f32 = mybir.dt.float32
    Exp = mybir.ActivationFunctionType.Exp

    nc = bacc.Bacc("TRN2", target_bir_lowering=False, debug=False)

    x_d = nc.dram_tensor("x", [6, 128, S], f16, kind="ExternalInput")
    wq_d = nc.dram_tensor("wq", [8, 128, 6, 128], f16, kind="ExternalInput")
    wk_d = nc.dram_tensor("wk", [8, 128, 6, 128], f16, kind="ExternalInput")
    wv_d = nc.dram_tensor("wv", [128, 6, 768], f16, kind="ExternalInput")
    wo_d = nc.dram_tensor("wo", [128, 6, 768], f16, kind="ExternalInput")
    bb_d = nc.dram_tensor("bb", [128, 22], f32, kind="ExternalInput")
    y_d = nc.dram_tensor("y", [D, S], f32, kind="ExternalOutput")

    with tile.TileContext(nc) as tc:
        with (
            tc.tile_pool(name="persist", bufs=1) as persist,
            tc.tile_pool(name="u", bufs=13) as upool,
            tc.tile_pool(name="ystage", bufs=3) as ypool,
            tc.tile_pool(name="nrm", bufs=2) as nrm,
            tc.tile_pool(name="psum", bufs=1, space=bass.MemorySpace.PSUM) as psum,
        ):
            wq_all = persist.tile([128, 8, 6, 128], f16, tag="wq", name="wq_all")
            wk_all = persist.tile([128, 8, 6, 128], f16, tag="wk", name="wk_all")
            wv_all = persist.tile([128, 6, 768], f16, tag="wv", name="wv_all")
            wo_all = persist.tile([128, 6, 768], f16, tag="wo", name="wo_all")
            wq_pk = lambda p, k: wq_all[:, p, k, :]
            wk_pk = lambda p, k: wk_all[:, p, k, :]
            wv_sb = [wv_all[:, k, :] for k in range(6)]
            wo_sb = [wo_all[:, k, :] for k in range(6)]
            xT_all = persist.tile([128, 6, 1024], f16, tag="xT", name="xT_all")
            xT_sb = [xT_all[:, k, :] for k in range(6)]
            qT_sb = [persist.tile([128, 1024], f16, tag=f"qT{p}", name=f"qT{p}") for p in range(8)]
            kT_sb = [persist.tile([128, 1024], f16, tag=f"kT{p}", name=f"kT{p}") for p in range(8)]
            # v_sb[m]: s_k chunk m on partitions; free = 16 head blocks of
            # 49 cols: [ones | 48 dims], head hb at cols 49*hb.
            v_sb = [persist.tile([128, 784], f16, tag=f"v{m}", name=f"v{m}") for m in range(8)]
            # ao[c]: s_q chunk c on partitions; free = 768 dims natural order
            ao_sb = [persist.tile([128, 768], f16, tag=f"ao{c}", name=f"ao{c}") for c in range(8)]
            # aoT[j]: dims chunk j on partitions; free = 1024 s_q
            aoT_sb = [persist.tile([128, 1024], f16, tag=f"aoT{j}", name=f"aoT{j}") for j in range(6)]
            bb_sb = persist.tile([128, 22], f32, tag="bb", name="bb_sb")
            zb_sb = persist.tile([128, 1], f32, tag="zb", name="zb_sb")
            bq_sb = bb_sb[:, 0:8]
            bk_sb = bb_sb[:, 8:16]
            bo_sb = bb_sb[:, 16:22]

            # ---- loads: bb rides the idle sync queue; pair-0 q/k weights
            # and the first x half lead the gpsimd stream so the first
            # projection group is DMA-fed from ~4us; SWDGE descriptor-gen
            # time on the Pool engine is the startup serializer, so bulk
            # tails (pairs 1-7) go as one DMA per tensor.
            nc.sync.dma_start(out=bb_sb[:], in_=bb_d[:])
            # pair-0 weights ride the two HWDGE queues (625ns overhead, no
            # Pool descriptor-gen) so the x halves lead the gpsimd stream
            nc.sync.dma_start(out=wq_all[:, 0, :, :], in_=wq_d[0])
            nc.scalar.dma_start(out=wk_all[:, 0, :, :], in_=wk_d[0])
            nc.gpsimd.dma_start(
                out=xT_all[:, 0:3, :], in_=x_d[0:3].rearrange("k p c -> p k c")
            )
            nc.gpsimd.dma_start(
                out=xT_all[:, 3:6, :], in_=x_d[3:6].rearrange("k p c -> p k c")
            )
            nc.gpsimd.dma_start(
                out=wq_all[:, 1:8, :, :], in_=wq_d[1:8].rearrange("q p k c -> p q k c")
            )
            nc.gpsimd.dma_start(
                out=wk_all[:, 1:8, :, :], in_=wk_d[1:8].rearrange("q p k c -> p q k c")
            )
            nc.gpsimd.dma_start(out=wv_all[:], in_=wv_d[:])
            nc.gpsimd.dma_start(out=wo_all[:], in_=wo_d[:])

            nc.gpsimd.memset(zb_sb[:], 0.0)
            for m in range(8):
                # softmax-denominator ones column of every head block
                vb = v_sb[m][:].rearrange("p (s c) -> p s c", c=49)
                nc.gpsimd.memset(vb[:, :, 0:1], 1.0)

            def qk_proj_group(p, which, n):
                """One [128,512] psum group: 6 matmuls + biased evacuation."""
                wpk, bsb, dst = (
                    (wq_pk, bq_sb, qT_sb) if which == "q" else (wk_pk, bk_sb, kT_sb)
                )
                ps = psum.tile([128, 512], f32, tag="mm", bufs=2, name="ps_mm_t")
                for k in range(6):
                    nc.tensor.matmul(
                        ps[:],
                        lhsT=wpk(p, k),
                        rhs=xT_sb[k][:, ts(n, 512)],
                        start=(k == 0),
                        stop=(k == 5),
                    )
                nc.vector.tensor_scalar_add(
                    dst[p][:, ts(n, 512)], ps[:], bsb[:, p : p + 1]
                )

            def v_proj_group(m, n):
                """One [128,384] psum group (8 heads) + strided evacuation."""
                ps = psum.tile([128, 512], f32, tag="mm", bufs=2, name="ps_mm_t")
                for k in range(6):
                    nc.tensor.matmul(
                        ps[:, 0:384],
                        lhsT=xT_sb[k][:, ts(m, 128)],
                        rhs=wv_sb[k][:, ts(n, 384)],
                        start=(k == 0),
                        stop=(k == 5),
                    )
                # psum [128, 8, 48] -> v_sb[m] cols 49*hb+1 .. +49, hb = 8n..
                src = ps[:, 0:384].rearrange("p (s c) -> p s c", c=48)
                dstv = v_sb[m][:, 392 * n : 392 * n + 392].rearrange(
                    "p (s c) -> p s c", c=49
                )
                nc.vector.tensor_copy(dstv[:, :, 1:49], src[:])

            # ---- deadline-scheduled projection FIFO --------------------
            # entries: (deadline_global_step, emit_fn). Global steps are
            # pair*8 + m over the attention loop (64 steps).
            proj_fifo = []

            def fifo_push(deadline, fn):
                proj_fifo.append((deadline, fn))

            emitted = [0]  # cumulative count

            def drain_proj(gstep, boost=0):
                """Emit everything due at gstep, plus keep an even-spread
                floor so late pairs still have PE fill-in work. `boost`
                allows pulling extra work into exp-wait-prone steps."""
                total = emitted[0] + len(proj_fifo)
                floor = -(-total * (gstep + 1) // 64) if gstep < 64 else total
                floor += boost
                while proj_fifo and (
                    proj_fifo[0][0] <= gstep or emitted[0] < floor
                ):
                    _, fn = proj_fifo.pop(0)
                    fn()
                    emitted[0] += 1

            # av slot bookkeeping: slot = 2*c + h, bank0 = slots 0..7
            def av_chunks(p, m, av):
                for c in range(8):
                    for h in range(2):
                        slot = 2 * c + h
                        nc.tensor.matmul(
                            av[:, 64 * slot : 64 * slot + 49],
                            lhsT=u_tiles[p][m][:, 1024 * h + 128 * c : 1024 * h + 128 * c + 128],
                            rhs=v_sb[m][:, 49 * (2 * p + h) : 49 * (2 * p + h) + 49],
                            start=(m == 0 and slot in (0, 8)),
                            stop=(m == 7 and slot in (7, 15)),
                        )

            def normalize(p, av, tail_js=()):
                """Divide by the denominator slot (GPSIMD cannot read PSUM,
                so all 16 muls run on DVE); for the last pair the dependent
                j-transposes are interleaved per s_q chunk."""
                rc = nrm.tile([128, 16], f32, tag="rc", name="rc_t")
                av_blk = av[:].rearrange("p (s c) -> p s c", c=64)
                nc.vector.reciprocal(rc[:], av_blk[:, :, 0:1])
                for c in range(8):
                    for h in range(2):
                        slot = 2 * c + h
                        hb = 2 * p + h
                        nc.vector.tensor_scalar_mul(
                            ao_sb[c][:, 48 * hb : 48 * hb + 48],
                            av[:, 64 * slot + 1 : 64 * slot + 49],
                            rc[:, slot : slot + 1],
                        )
                    for j in tail_js:
                        nc.sync.dma_start(
                            out=aoT_sb[j][:, ts(c, 128)],
                            in_=ao_sb[c][:, ts(j, 128)],
                            transpose=True,
                        )

            def av_chunks7(m, tiles):
                """Pair 7's AV into the two (FIFO-idle by then) mm slots so
                it runs during pair 7's own m-steps instead of serializing
                behind normalize(6) in the epilogue."""
                for c in range(8):
                    for h in range(2):
                        slot = 2 * c + h
                        t = tiles[slot // 8]
                        col = 64 * (slot % 8)
                        nc.tensor.matmul(
                            t[:, col : col + 49],
                            lhsT=u_tiles[7][m][:, 1024 * h + 128 * c : 1024 * h + 128 * c + 128],
                            rhs=v_sb[m][:, 49 * (14 + h) : 49 * (14 + h) + 49],
                            start=(m == 0 and slot % 8 == 0),
                            stop=(m == 7 and slot % 8 == 7),
                        )

            def normalize7(tiles):
                rc = nrm.tile([128, 16], f32, tag="rc", name="rc_t")
                for t_i, t in enumerate(tiles):
                    blk = t[:].rearrange("p (s c) -> p s c", c=64)
                    nc.vector.reciprocal(rc[:, 8 * t_i : 8 * t_i + 8], blk[:, :, 0:1])
                for c in range(8):
                    for h in range(2):
                        slot = 2 * c + h
                        t = tiles[slot // 8]
                        col = 64 * (slot % 8)
                        hb = 14 + h
                        nc.vector.tensor_scalar_mul(
                            ao_sb[c][:, 48 * hb : 48 * hb + 48],
                            t[:, col + 1 : col + 49],
                            rc[:, slot : slot + 1],
                        )
                    nc.sync.dma_start(
                        out=aoT_sb[5][:, ts(c, 128)],
                        in_=ao_sb[c][:, ts(5, 128)],
                        transpose=True,
                    )

            def transpose_j(j):
                for c in range(8):
                    nc.sync.dma_start(
                        out=aoT_sb[j][:, ts(c, 128)],
                        in_=ao_sb[c][:, ts(j, 128)],
                        transpose=True,
                    )

            # dim block j is ready once pair q >= JREADY[j] has normalized
            JREADY = {0: 1, 1: 2, 2: 3, 3: 5, 4: 6, 5: 7}

            def scores_exp(p, m):
                # scA = [A-n0 | B-n0], scB = [A-n1 | B-n1]
                scA = psum.tile([128, 1024], f32, tag="scA", name="scA_t")
                scB = psum.tile([128, 1024], f32, tag="scB", name="scB_t")
                u = upool.tile([128, 2048], f16, tag="u", name="u_t")
                for n, sc in ((0, scA), (1, scB)):
                    nc.tensor.matmul(
                        sc[:, 0:512],
                        lhsT=kT_sb[p][0:48, ts(m, 128)],
                        rhs=qT_sb[p][0:48, ts(n, 512)],
                        start=True,
                        stop=True,
                        tile_position=(0, 0),
                    )
                    nc.tensor.matmul(
                        sc[:, 512:1024],
                        lhsT=kT_sb[p][64:112, ts(m, 128)],
                        rhs=qT_sb[p][64:112, ts(n, 512)],
                        start=True,
                        stop=True,
                        tile_position=(64, 0),
                    )
                    # u view: [128, 2 heads, 1024 s_q] -> n-th 512 of each
                    uv = u[:].rearrange("p (h n) -> p h n", n=1024)
                    scv = sc[:].rearrange("p (h n) -> p h n", n=512)
                    nc.scalar.activation(
                        uv[:, :, 512 * n : 512 * n + 512], scv[:], Exp, bias=zb_sb[:]
                    )
                return u

            def out_proj():
                # runs in the epilogue when the attention psum banks are
                # free: j-groups rotate through the scA/scB/av tags so three
                # [128,1024] groups are in flight. Every group needs aoT[5],
                # which is only transposed after pair 7 normalizes — so the
                # k<=4 matmuls of the first three groups are emitted first
                # and the k=5 closers deferred, hiding the transpose wait.
                # Stores ride the idle sync queue (HWDGE) per 512-col half.
                tags = ["scA", "scB", "av"]
                pss, yss = [], []
                for j in range(3):
                    ps = psum.tile([128, 1024], f32, tag=tags[j], name="ps_o_t")
                    pss.append(ps)
                    for n in range(2):
                        for k in range(4):
                            nc.tensor.matmul(
                                ps[:, ts(n, 512)],
                                lhsT=wo_sb[k][:, ts(j, 128)],
                                rhs=aoT_sb[k][:, ts(n, 512)],
                                start=(k == 0),
                                stop=False,
                            )
                # k=4 staged after the j4 transposes land, k=5 after j5's
                for j in range(3):
                    for n in range(2):
                        nc.tensor.matmul(
                            pss[j][:, ts(n, 512)],
                            lhsT=wo_sb[4][:, ts(j, 128)],
                            rhs=aoT_sb[4][:, ts(n, 512)],
                            start=False,
                            stop=False,
                        )

                def finish(j, ps):
                    ys = ypool.tile([128, 1024], f32, tag="ys", bufs=4, name="ys_t")
                    for n in range(2):
                        nc.tensor.matmul(
                            ps[:, ts(n, 512)],
                            lhsT=wo_sb[5][:, ts(j, 128)],
                            rhs=aoT_sb[5][:, ts(n, 512)],
                            start=False,
                            stop=True,
                        )
                        nc.vector.tensor_scalar_add(
                            ys[:, ts(n, 512)], ps[:, ts(n, 512)], bo_sb[:, j : j + 1]
                        )
                        nc.sync.dma_start(
                            out=y_d[ts(j, 128), ts(n, 512)], in_=ys[:, ts(n, 512)]
                        )

                for j in range(3):
                    finish(j, pss[j])
                for j in range(3, 6):
                    ps = psum.tile([128, 1024], f32, tag=tags[j % 3], name="ps_o_t")
                    for n in range(2):
                        for k in range(6):
                            nc.tensor.matmul(
                                ps[:, ts(n, 512)],
                                lhsT=wo_sb[k][:, ts(j, 128)],
                                rhs=aoT_sb[k][:, ts(n, 512)],
                                start=(k == 0),
                                stop=(k == 5),
                            )
                        ys = ypool.tile([128, 512], f32, tag="ys2", bufs=6, name="ys2_t")
                        nc.vector.tensor_scalar_add(
                            ys[:], ps[:, ts(n, 512)], bo_sb[:, j : j + 1]
                        )
                        nc.sync.dma_start(out=y_d[ts(j, 128), ts(n, 512)], in_=ys[:])

            u_tiles = {}
            for _rep in range(reps):
                # prologue: pair 0's q/k, n0 halves first (interleaved at
                # matmul level so the two groups ride the DMA chunk stream)
                psq = psum.tile([128, 512], f32, tag="mm", bufs=2, name="ps_mm_t")
                psk = psum.tile([128, 512], f32, tag="mm", bufs=2, name="ps_mm_t")
                for k in range(6):
                    nc.tensor.matmul(
                        psq[:], lhsT=wq_pk(0, k), rhs=xT_sb[k][:, 0:512],
                        start=(k == 0), stop=(k == 5),
                    )
                    nc.tensor.matmul(
                        psk[:], lhsT=wk_pk(0, k), rhs=xT_sb[k][:, 0:512],
                        start=(k == 0), stop=(k == 5),
                    )
                nc.vector.tensor_scalar_add(qT_sb[0][:, 0:512], psq[:], bq_sb[:, 0:1])
                nc.vector.tensor_scalar_add(kT_sb[0][:, 0:512], psk[:], bk_sb[:, 0:1])
                qk_proj_group(0, "q", 1)
                qk_proj_group(0, "k", 1)

                # FIFO: qk(p) due before pair p starts; v(m) due before
                # AV(0,m) which runs at pair-1 step 4+m//2 (minus 1 slack)
                proj_fifo.clear()
                emitted[0] = 0
                for n in range(2):
                    fifo_push(7, lambda n=n: qk_proj_group(1, "q", n))
                    fifo_push(7, lambda n=n: qk_proj_group(1, "k", n))
                for m in range(8):
                    for n in range(2):
                        fifo_push(7 + m // 2, lambda m=m, n=n: v_proj_group(m, n))
                for p in range(2, 8):
                    for n in range(2):
                        fifo_push(8 * p - 1, lambda p=p, n=n: qk_proj_group(p, "q", n))
                        fifo_push(8 * p - 1, lambda p=p, n=n: qk_proj_group(p, "k", n))
                proj_fifo.sort(key=lambda e: e[0])

                u_tiles.clear()
                u_tiles.update({p: {} for p in range(8)})
                av_cur = None  # av psum tile for the pair whose AV is running
                for p in range(8):
                    if p >= 1:
                        av_cur = psum.tile([128, 1024], f32, tag="av", name="av_t")
                    av7 = None
                    for m in range(8):
                        u_tiles[p][m] = scores_exp(p, m)
                        drain_proj(8 * p + m, boost=1 if m < 2 else 0)
                        if p >= 1 and m < 2:
                            av_chunks(p - 1, 2 * m, av_cur)
                            av_chunks(p - 1, 2 * m + 1, av_cur)
                        if p >= 1 and 2 <= m < 6:
                            av_chunks(p - 1, m + 2, av_cur)
                        if p >= 1 and m == 6:
                            normalize(p - 1, av_cur)
                            for j, q in JREADY.items():
                                if q == p - 1:
                                    transpose_j(j)
                        if p == 7 and m >= 2:
                            if av7 is None:
                                av7 = [
                                    psum.tile([128, 512], f32, tag="mm", bufs=2, name="ps_mm_t")
                                    for _ in range(2)
                                ]
                            av_chunks7(m - 2, av7)
                # epilogue: pair 7's last AV chunks, normalize + j5
                # transposes (interleaved per chunk), then the output proj
                av_chunks7(6, av7)
                av_chunks7(7, av7)
                normalize7(av7)
                out_proj()

    nc.compile()
    return nc


def _get_nc(reps=1, skip_attention=False):
    key = f"nc{reps}_{skip_attention}"
    if key not in _CACHE:
        _CACHE[key] = _build_nc(reps, skip_attention)
    return _CACHE[key]


def _perm_cols(w):
    """[768, 768] -> [768, 1024]: head-pair column layout, zero padded."""
    out = np.zeros((D, 1024), np.float32)
    for p in range(PAIRS):
        out[:, 128 * p : 128 * p + 48] = w[:, 96 * p : 96 * p + 48]
        out[:, 128 * p + 64 : 128 * p + 112] = w[:, 96 * p + 48 : 96 * p + 96]
    return out


def _pack_bias_pairs(b):
    """[768] -> [128, 8]: per-pair per-partition bias columns."""
    t = np.zeros((128, PAIRS), np.float32)
    for p in range(PAIRS):
        t[0:48, p] = b[96 * p : 96 * p + 48]
        t[64:112, p] = b[96 * p + 48 : 96 * p + 96]
    return t


def _swizzle(w, nt, cols):
    """[nt*128, cols] -> [128, nt, cols] partition-major SBUF image."""
    return np.ascontiguousarray(w.reshape(nt, 128, cols).transpose(1, 0, 2))


def kernel(x, Wq, bq, Wk, bk, Wv, bv, Wo, bo, _trace=False):
    global LAST_RESULTS, LAST_IN_MAPS
    from concourse.bass_utils import run_bass_kernel_spmd

    x = np.asarray(x, np.float32)
    Wq = np.asarray(Wq, np.float32)
    Wk = np.asarray(Wk, np.float32)
    Wv = np.asarray(Wv, np.float32)
    Wo = np.asarray(Wo, np.float32)
    bq = np.asarray(bq, np.float32)
    bk = np.asarray(bk, np.float32)
    bv = np.asarray(bv, np.float32)
    bo = np.asarray(bo, np.float32)

    s = np.float32(1.0 / math.sqrt(HD))
    # [128, 6, 1024] -> pair-outermost [8, 128, 6, 128] so each pair is one
    # contiguous DMA (and pairs 1-7 a single bulk DMA)
    wq_p = np.ascontiguousarray(
        _swizzle(_perm_cols(Wq * s).astype(np.float16), 6, 1024)
        .reshape(128, 6, 8, 128)
        .transpose(2, 0, 1, 3)
    )
    wk_p = np.ascontiguousarray(
        _swizzle(_perm_cols(Wk).astype(np.float16), 6, 1024)
        .reshape(128, 6, 8, 128)
        .transpose(2, 0, 1, 3)
    )
    wv_p = _swizzle(Wv.astype(np.float16), 6, 768)
    wo_p = _swizzle(Wo.astype(np.float16), 6, 768)
    bb = np.zeros((128, 22), np.float32)
    bb[:, 0:8] = _pack_bias_pairs(bq * s)
    bb[:, 8:16] = _pack_bias_pairs(bk)
    bo_eff = bo + bv @ Wo
    bb[:, 16:22] = bo_eff.reshape(6, 128).T

    x16 = x.astype(np.float16)  # [B, S, D]

    shared = {"wq": wq_p, "wk": wk_p, "wv": wv_p, "wo": wo_p, "bb": bb}
    in_maps = [
        dict(shared, x=np.ascontiguousarray(x16[i].T).reshape(6, 128, 1024))
        for i in range(N_CORES)
    ]
    LAST_IN_MAPS = in_maps

    nc = _get_nc()
    try:
        res = run_bass_kernel_spmd(
            nc, in_maps, core_ids=list(range(N_CORES)), trace=_trace
        )
    except ModuleNotFoundError:
        # no axon NTFF profiling hook in this container
        res = run_bass_kernel_spmd(nc, in_maps, core_ids=list(range(N_CORES)))
    LAST_RESULTS = res

    y = np.stack([res.results[i]["y"].T for i in range(N_CORES)])  # [B, S, D]
    return np.ascontiguousarray(y.astype(np.float32))


# revision 86
# speedup vs baseline: 2.5434x; 2.5434x over previous
"""Multi-head attention TRN2 kernel (v3: transposed-AV + pipelined pairs).

Problem: B=8, S=1024, D=768, H=16, Hd=48 MHA (dense_transformer).
Sharding: pure data parallel — one batch element per NeuronCore (8 cores).

Per-core device kernel:
  xT  [D, S]   host-pre-transposed, plain chunk DMAs
  qT  [D, S]   = (Wq/sqrt(Hd))^T @ xT + bq/sqrt(Hd)   (head-pair col layout)
  kT  [D, S]   = Wk^T @ xT + bk                        (head-pair col layout)
  v   [S, D]   = x @ Wv      stored per-head as [ones | 48 dims] 49-col blocks
  per pair p = heads (2p, 2p+1), per s_k chunk m:
    scoresT[S_k, S_q] = kT_h^T-contract qT_h  (K=48, two heads packed per PE
                        pass via row tile_position); scA holds the s_q-n0
                        half of both heads, scB the n1 half, so exp(n0) can
                        fire before the n1 projections even exist
    U = exp(scoresT)   (ACT engine; no max subtraction; scores ~ N(0,1))
  AV in the TRANSPOSED orientation: for s_q chunk c, head h, accumulate
    av[s_q, 0:49] += U_h[s_k chunk m, 128c:+128]^T @ [1 | v_h][s_k chunk m]
  so each AV matmul is M=128 (s_q), K=128 (s_k), N=49 — the cost model
  charges N only: AV is 50176 PE rows instead of v1's 131072. Slot 0 is the
  softmax denominator (ones column) -> normalization is a per-partition
  tensor_scalar multiply, no partition broadcast.
  AV(p-1) chunks execute during pair p's m-steps 4..7 (software pipeline one
  pair back) so exp(p-1,m) -> AV(p-1,m) handoff and the av-psum WAR against
  normalize(p-2) both have a full pair of slack.
  ao[c] [S_q chunk, D] fp16 (natural dim order) -> DMA-transpose (sync
  queue HWDGE, SBUF->SBUF [128,128] blocks, emitted as soon as every pair
  covering dim block j has normalized) -> aoT [D, S] dense
  yT [D, S] = Wo^T @ aoT + (bo + bv @ Wo)   (dense 6x6 contraction)

Layout invariants driven by hardware rules:
- engine SBUF/PSUM access patterns must start at partition 0/32/64/96, so
  qT/kT keep the 2-heads-per-128-partition pair layout (rows 0:48, 64:112)
- a matmul start=True marks pending-zero for its WHOLE psum bank (2KB zero
  region); the av tile runs one multi-slot accumulation group per bank
  (start=True on the bank's first matmul, each slot's first write stores,
  later writes accumulate, stop=True on the bank's last matmul)
- psum budget exactly 8 banks: scA(2) + scB(2) + av(2) + mm 2x[128,512](2)
- projection groups are deadline-scheduled into the attention m-steps with
  an even-spread floor so the PE has fill-in work under exp for ALL pairs
- HWDGE DMA transposes go on the otherwise-idle sync queue; bulk DMAs on
  gpsimd SWDGE, ordered (wq_k, wk_k, x_k) interleaved so the first q/k
  projection group is DMA-fed chunk by chunk from ~1.5us
"""

import math

import numpy as np

B, S, D = 8, 1024, 768
H, HD = 16, 48
PAIRS = H // 2
N_CORES = 8

_CACHE = {}
LAST_RESULTS = None
LAST_IN_MAPS = None


def _build_nc(reps=1, skip_attention=False):
    import concourse.bass as bass
    import concourse.mybir as mybir
    import concourse.tile as tile
    from concourse import bacc
    from concourse.bass import ts

    f16 = mybir.dt.float16
    f32 = mybir.dt.float32
    Exp = mybir.ActivationFunctionType.Exp
    Copy = mybir.ActivationFunctionType.Copy

    nc = bacc.Bacc("TRN2", target_bir_lowering=False, debug=False)

    x_d = nc.dram_tensor("x", [6, 128, S], f16, kind="ExternalInput")
    wq_d = nc.dram_tensor("wq", [8, 128, 6, 128], f16, kind="ExternalInput")
    wk_d = nc.dram_tensor("wk", [8, 128, 6, 128], f16, kind="ExternalInput")
    wv_d = nc.dram_tensor("wv", [128, 6, 768], f16, kind="ExternalInput")
    wo_d = nc.dram_tensor("wo", [128, 6, 768], f16, kind="ExternalInput")
    bb_d = nc.dram_tensor("bb", [128, 22], f32, kind="ExternalInput")
    y_d = nc.dram_tensor("y", [D, S], f32, kind="ExternalOutput")

    with tile.TileContext(nc) as tc:
        with (
            tc.tile_pool(name="persist", bufs=1) as persist,
            tc.tile_pool(name="u", bufs=13) as upool,
            tc.tile_pool(name="ystage", bufs=3) as ypool,
            tc.tile_pool(name="nrm", bufs=2) as nrm,
            tc.tile_pool(name="psum", bufs=1, space=bass.MemorySpace.PSUM) as psum,
        ):
            wq_all = persist.tile([128, 8, 6, 128], f16, tag="wq", name="wq_all")
            wk_all = persist.tile([128, 8, 6, 128], f16, tag="wk", name="wk_all")
            wv_all = persist.tile([128, 6, 768], f16, tag="wv", name="wv_all")
            wo_all = persist.tile([128, 6, 768], f16, tag="wo", name="wo_all")
            wq_pk = lambda p, k: wq_all[:, p, k, :]
            wk_pk = lambda p, k: wk_all[:, p, k, :]
            wv_sb = [wv_all[:, k, :] for k in range(6)]
            wo_sb = [wo_all[:, k, :] for k in range(6)]
            xT_all = persist.tile([128, 6, 1024], f16, tag="xT", name="xT_all")
            xT_sb = [xT_all[:, k, :] for k in range(6)]
            qT_sb = [persist.tile([128, 1024], f16, tag=f"qT{p}", name=f"qT{p}") for p in range(8)]
            kT_sb = [persist.tile([128, 1024], f16, tag=f"kT{p}", name=f"kT{p}") for p in range(8)]
            # v_sb[m]: s_k chunk m on partitions; free = 16 head blocks of
            # 49 cols: [ones | 48 dims], head hb at cols 49*hb.
            v_sb = [persist.tile([128, 784], f16, tag=f"v{m}", name=f"v{m}") for m in range(8)]
            # ao[c]: s_q chunk c on partitions; free = 768 dims natural order
            ao_sb = [persist.tile([128, 768], f16, tag=f"ao{c}", name=f"ao{c}") for c in range(8)]
            # aoT[j]: dims chunk j on partitions; free = 1024 s_q
            aoT_sb = [persist.tile([128, 1024], f16, tag=f"aoT{j}", name=f"aoT{j}") for j in range(6)]
            bb_sb = persist.tile([128, 22], f32, tag="bb", name="bb_sb")
            zb_sb = persist.tile([128, 1], f32, tag="zb", name="zb_sb")
            bq_sb = bb_sb[:, 0:8]
            bk_sb = bb_sb[:, 8:16]
            bo_sb = bb_sb[:, 16:22]

            # ---- loads: bb rides the idle sync queue; pair-0 q/k weights
            # and the first x half lead the gpsimd stream so the first
            # projection group is DMA-fed from ~4us; SWDGE descriptor-gen
            # time on the Pool engine is the startup serializer, so bulk
            # tails (pairs 1-7) go as one DMA per tensor.
            nc.sync.dma_start(out=bb_sb[:], in_=bb_d[:])
            # pair-0 weights ride the two HWDGE queues (625ns overhead, no
            # Pool descriptor-gen) so the x halves lead the gpsimd stream
            nc.sync.dma_start(out=wq_all[:, 0, :, :], in_=wq_d[0])
            nc.scalar.dma_start(out=wk_all[:, 0, :, :], in_=wk_d[0])
            nc.gpsimd.dma_start(
                out=xT_all[:, 0:3, :], in_=x_d[0:3].rearrange("k p c -> p k c")
            )
            nc.gpsimd.dma_start(
                out=xT_all[:, 3:6, :], in_=x_d[3:6].rearrange("k p c -> p k c")
            )
            nc.gpsimd.dma_start(
                out=wq_all[:, 1:8, :, :], in_=wq_d[1:8].rearrange("q p k c -> p q k c")
            )
            nc.gpsimd.dma_start(
                out=wk_all[:, 1:8, :, :], in_=wk_d[1:8].rearrange("q p k c -> p q k c")
            )
            nc.gpsimd.dma_start(out=wv_all[:], in_=wv_d[:])
            nc.gpsimd.dma_start(out=wo_all[:], in_=wo_d[:])

            nc.gpsimd.memset(zb_sb[:], 0.0)
            for m in range(8):
                # softmax-denominator ones column of every head block
                vb = v_sb[m][:].rearrange("p (s c) -> p s c", c=49)
                nc.gpsimd.memset(vb[:, :, 0:1], 1.0)

            def qk_proj_group(p, which, n):
                """One [128,512] psum group: 6 matmuls + biased evacuation."""
                wpk, bsb, dst = (
                    (wq_pk, bq_sb, qT_sb) if which == "q" else (wk_pk, bk_sb, kT_sb)
                )
                ps = psum.tile([128, 512], f32, tag="mm", bufs=2, name="ps_mm_t")
                for k in range(6):
                    nc.tensor.matmul(
                        ps[:],
                        lhsT=wpk(p, k),
                        rhs=xT_sb[k][:, ts(n, 512)],
                        start=(k == 0),
                        stop=(k == 5),
                    )
                nc.vector.tensor_scalar_add(
                    dst[p][:, ts(n, 512)], ps[:], bsb[:, p : p + 1]
                )

            def v_proj_group(m, n):
                """One [128,384] psum group (8 heads) + strided evacuation."""
                ps = psum.tile([128, 512], f32, tag="mm", bufs=2, name="ps_mm_t")
                for k in range(6):
                    nc.tensor.matmul(
                        ps[:, 0:384],
                        lhsT=xT_sb[k][:, ts(m, 128)],
                        rhs=wv_sb[k][:, ts(n, 384)],
                        start=(k == 0),
                        stop=(k == 5),
                    )
                # psum [128, 8, 48] -> v_sb[m] cols 49*hb+1 .. +49, hb = 8n..
                src = ps[:, 0:384].rearrange("p (s c) -> p s c", c=48)
                dstv = v_sb[m][:, 392 * n : 392 * n + 392].rearrange(
                    "p (s c) -> p s c", c=49
                )
                nc.vector.tensor_copy(dstv[:, :, 1:49], src[:])

            # ---- deadline-scheduled projection FIFO --------------------
            # entries: (deadline_global_step, emit_fn). Global steps are
            # pair*8 + m over the attention loop (64 steps).
            proj_fifo = []

            def fifo_push(deadline, fn):
                proj_fifo.append((deadline, fn))

            emitted = [0]  # cumulative count

            def drain_proj(gstep, boost=0):
                """Emit everything due at gstep, plus keep an even-spread
                floor so late pairs still have PE fill-in work. `boost`
                allows pulling extra work into exp-wait-prone steps."""
                total = emitted[0] + len(proj_fifo)
                floor = -(-total * (gstep + 1) // 64) if gstep < 64 else total
                floor += boost
                while proj_fifo and (
                    proj_fifo[0][0] <= gstep or emitted[0] < floor
                ):
                    _, fn = proj_fifo.pop(0)
                    fn()
                    emitted[0] += 1

            # av slot bookkeeping: slot = 2*c + h, bank0 = slots 0..7
            def av_chunks(p, m, av):
                for c in range(8):
                    for h in range(2):
                        slot = 2 * c + h
                        nc.tensor.matmul(
                            av[:, 64 * slot : 64 * slot + 49],
                            lhsT=u_tiles[p][m][:, 1024 * h + 128 * c : 1024 * h + 128 * c + 128],
                            rhs=v_sb[m][:, 49 * (2 * p + h) : 49 * (2 * p + h) + 49],
                            start=(m == 0 and slot in (0, 8)),
                            stop=(m == 7 and slot in (7, 15)),
                        )

            def normalize(p, av, tail_js=()):
                """Divide by the denominator slot (GPSIMD cannot read PSUM,
                so all 16 muls run on DVE); for the last pair the dependent
                j-transposes are interleaved per s_q chunk."""
                rc = nrm.tile([128, 16], f32, tag="rc", name="rc_t")
                av_blk = av[:].rearrange("p (s c) -> p s c", c=64)
                nc.vector.reciprocal(rc[:], av_blk[:, :, 0:1])
                for c in range(8):
                    for h in range(2):
                        slot = 2 * c + h
                        hb = 2 * p + h
                        nc.vector.tensor_scalar_mul(
                            ao_sb[c][:, 48 * hb : 48 * hb + 48],
                            av[:, 64 * slot + 1 : 64 * slot + 49],
                            rc[:, slot : slot + 1],
                        )
                    for j in tail_js:
                        nc.sync.dma_start(
                            out=aoT_sb[j][:, ts(c, 128)],
                            in_=ao_sb[c][:, ts(j, 128)],
                            transpose=True,
                        )

            def av_chunks7(m, tiles):
                """Pair 7's AV into the two (FIFO-idle by then) mm slots so
                it runs during pair 7's own m-steps instead of serializing
                behind normalize(6) in the epilogue."""
                for c in range(8):
                    for h in range(2):
                        slot = 2 * c + h
                        t = tiles[slot // 8]
                        col = 64 * (slot % 8)
                        nc.tensor.matmul(
                            t[:, col : col + 49],
                            lhsT=u_tiles[7][m][:, 1024 * h + 128 * c : 1024 * h + 128 * c + 128],
                            rhs=v_sb[m][:, 49 * (14 + h) : 49 * (14 + h) + 49],
                            start=(m == 0 and slot % 8 == 0),
                            stop=(m == 7 and slot % 8 == 7),
                        )

            def normalize7(tiles):
                rc = nrm.tile([128, 16], f32, tag="rc", name="rc_t")
                for t_i, t in enumerate(tiles):
                    blk = t[:].rearrange("p (s c) -> p s c", c=64)
                    nc.vector.reciprocal(rc[:, 8 * t_i : 8 * t_i + 8], blk[:, :, 0:1])
                # chunks 0-3 normalize on DVE, 4-7 on the (now idle) ACT
                # engine via Copy-with-scale, halving the serial mul chain;
                # each chunk's j5 transpose rides the queue whose engine
                # produced it, so the late dependency is implicit in-order
                for c in range(8):
                    for h in range(2):
                        slot = 2 * c + h
                        t = tiles[slot // 8]
                        col = 64 * (slot % 8)
                        hb = 14 + h
                        if c < 4:
                            nc.vector.tensor_scalar_mul(
                                ao_sb[c][:, 48 * hb : 48 * hb + 48],
                                t[:, col + 1 : col + 49],
                                rc[:, slot : slot + 1],
                            )
                        else:
                            nc.scalar.activation(
                                ao_sb[c][:, 48 * hb : 48 * hb + 48],
                                t[:, col + 1 : col + 49],
                                Copy,
                                bias=0.0,
                                scale=rc[:, slot : slot + 1],
                            )
                    eng = nc.sync
                    eng.dma_start(
                        out=aoT_sb[5][:, ts(c, 128)],
                        in_=ao_sb[c][:, ts(5, 128)],
                        transpose=True,
                    )

            def transpose_j(j):
                for c in range(8):
                    nc.sync.dma_start(
                        out=aoT_sb[j][:, ts(c, 128)],
                        in_=ao_sb[c][:, ts(j, 128)],
                        transpose=True,
                    )

            # dim block j is ready once pair q >= JREADY[j] has normalized
            JREADY = {0: 1, 1: 2, 2: 3, 3: 5, 4: 6, 5: 7}

            def scores_exp(p, m):
                # scA = [A-n0 | B-n0], scB = [A-n1 | B-n1]
                scA = psum.tile([128, 1024], f32, tag="scA", name="scA_t")
                scB = psum.tile([128, 1024], f32, tag="scB", name="scB_t")
                u = upool.tile([128, 2048], f16, tag="u", name="u_t")
                for n, sc in ((0, scA), (1, scB)):
                    nc.tensor.matmul(
                        sc[:, 0:512],
                        lhsT=kT_sb[p][0:48, ts(m, 128)],
                        rhs=qT_sb[p][0:48, ts(n, 512)],
                        start=True,
                        stop=True,
                        tile_position=(0, 0),
                    )
                    nc.tensor.matmul(
                        sc[:, 512:1024],
                        lhsT=kT_sb[p][64:112, ts(m, 128)],
                        rhs=qT_sb[p][64:112, ts(n, 512)],
                        start=True,
                        stop=True,
                        tile_position=(64, 0),
                    )
                    # u view: [128, 2 heads, 1024 s_q] -> n-th 512 of each
                    uv = u[:].rearrange("p (h n) -> p h n", n=1024)
                    scv = sc[:].rearrange("p (h n) -> p h n", n=512)
                    nc.scalar.activation(
                        uv[:, :, 512 * n : 512 * n + 512], scv[:], Exp, bias=zb_sb[:]
                    )
                return u

            def out_proj():
                # runs in the epilogue when the attention psum banks are
                # free: j-groups rotate through the scA/scB/av tags so three
                # [128,1024] groups are in flight. Every group needs aoT[5],
                # which is only transposed after pair 7 normalizes — so the
                # k<=4 matmuls of the first three groups are emitted first
                # and the k=5 closers deferred, hiding the transpose wait.
                # Stores ride the idle sync queue (HWDGE) per 512-col half.
                tags = ["scA", "scB", "av"]
                pss, yss = [], []
                for j in range(3):
                    ps = psum.tile([128, 1024], f32, tag=tags[j], name="ps_o_t")
                    pss.append(ps)
                    for n in range(2):
                        for k in range(4):
                            nc.tensor.matmul(
                                ps[:, ts(n, 512)],
                                lhsT=wo_sb[k][:, ts(j, 128)],
                                rhs=aoT_sb[k][:, ts(n, 512)],
                                start=(k == 0),
                                stop=False,
                            )
                # k=4 staged after the j4 transposes land, k=5 after j5's
                for j in range(3):
                    for n in range(2):
                        nc.tensor.matmul(
                            pss[j][:, ts(n, 512)],
                            lhsT=wo_sb[4][:, ts(j, 128)],
                            rhs=aoT_sb[4][:, ts(n, 512)],
                            start=False,
                            stop=False,
                        )

                def finish(j, ps):
                    ys = ypool.tile([128, 1024], f32, tag="ys", bufs=4, name="ys_t")
                    for n in range(2):
                        nc.tensor.matmul(
                            ps[:, ts(n, 512)],
                            lhsT=wo_sb[5][:, ts(j, 128)],
                            rhs=aoT_sb[5][:, ts(n, 512)],
                            start=False,
                            stop=True,
                        )
                        nc.vector.tensor_scalar_add(
                            ys[:, ts(n, 512)], ps[:, ts(n, 512)], bo_sb[:, j : j + 1]
                        )
                        nc.sync.dma_start(
                            out=y_d[ts(j, 128), ts(n, 512)], in_=ys[:, ts(n, 512)]
                        )

                for j in range(3):
                    finish(j, pss[j])
                # j3 runs its halves through the mm slots, which free as
                # soon as normalize7 has read pair-7's AV — about 2us before
                # j0's evacuations release the scA tag for reuse
                for j in range(3, 6):
                    psn = []
                    if j == 3:
                        for n in range(2):
                            psn.append(
                                psum.tile([128, 512], f32, tag="mm", bufs=2, name="ps_mm_t")[:]
                            )
                    else:
                        psj = psum.tile(
                            [128, 1024], f32, tag=("scA" if j == 4 else "scB"), name="ps_o_t"
                        )
                        psn = [psj[:, 0:512], psj[:, 512:1024]]
                    for n in range(2):
                        for k in range(6):
                            nc.tensor.matmul(
                                psn[n],
                                lhsT=wo_sb[k][:, ts(j, 128)],
                                rhs=aoT_sb[k][:, ts(n, 512)],
                                start=(k == 0),
                                stop=(k == 5),
                            )
                        ys = ypool.tile([128, 512], f32, tag="ys2", bufs=6, name="ys2_t")
                        nc.vector.tensor_scalar_add(
                            ys[:], psn[n], bo_sb[:, j : j + 1]
                        )
                        nc.sync.dma_start(out=y_d[ts(j, 128), ts(n, 512)], in_=ys[:])

            u_tiles = {}
            for _rep in range(reps):
                # prologue: pair 0's q/k, n0 halves first (interleaved at
                # matmul level so the two groups ride the DMA chunk stream)
                psq = psum.tile([128, 512], f32, tag="mm", bufs=2, name="ps_mm_t")
                psk = psum.tile([128, 512], f32, tag="mm", bufs=2, name="ps_mm_t")
                for k in range(6):
                    nc.tensor.matmul(
                        psq[:], lhsT=wq_pk(0, k), rhs=xT_sb[k][:, 0:512],
                        start=(k == 0), stop=(k == 5),
                    )
                    nc.tensor.matmul(
                        psk[:], lhsT=wk_pk(0, k), rhs=xT_sb[k][:, 0:512],
                        start=(k == 0), stop=(k == 5),
                    )
                nc.vector.tensor_scalar_add(qT_sb[0][:, 0:512], psq[:], bq_sb[:, 0:1])
                nc.vector.tensor_scalar_add(kT_sb[0][:, 0:512], psk[:], bk_sb[:, 0:1])
                qk_proj_group(0, "q", 1)
                qk_proj_group(0, "k", 1)

                # FIFO: qk(p) due before pair p starts; v(m) due before
                # AV(0,m) which runs at pair-1 step 4+m//2 (minus 1 slack)
                proj_fifo.clear()
                emitted[0] = 0
                for n in range(2):
                    fifo_push(7, lambda n=n: qk_proj_group(1, "q", n))
                    fifo_push(7, lambda n=n: qk_proj_group(1, "k", n))
                for m in range(8):
                    for n in range(2):
                        fifo_push(7 + m // 2, lambda m=m, n=n: v_proj_group(m, n))
                for p in range(2, 8):
                    for n in range(2):
                        fifo_push(8 * p - 1, lambda p=p, n=n: qk_proj_group(p, "q", n))
                        fifo_push(8 * p - 1, lambda p=p, n=n: qk_proj_group(p, "k", n))
                proj_fifo.sort(key=lambda e: e[0])

                u_tiles.clear()
                u_tiles.update({p: {} for p in range(8)})
                av_cur = None  # av psum tile for the pair whose AV is running
                for p in range(8):
                    if p >= 1:
                        av_cur = psum.tile([128, 1024], f32, tag="av", name="av_t")
                    av7 = None
                    for m in range(8):
                        u_tiles[p][m] = scores_exp(p, m)
                        drain_proj(8 * p + m, boost=1 if m < 2 else 0)
                        if p >= 1 and m < 2:
                            av_chunks(p - 1, 2 * m, av_cur)
                            av_chunks(p - 1, 2 * m + 1, av_cur)
                        if p >= 1 and 2 <= m < 6:
                            av_chunks(p - 1, m + 2, av_cur)
                        if p >= 1 and m == 6:
                            normalize(p - 1, av_cur)
                            for j, q in JREADY.items():
                                if q == p - 1:
                                    transpose_j(j)
                        if p == 7 and m >= 2:
                            if av7 is None:
                                av7 = [
                                    psum.tile([128, 512], f32, tag="mm", bufs=2, name="ps_mm_t")
                                    for _ in range(2)
                                ]
                            av_chunks7(m - 2, av7)
                # epilogue: pair 7's last AV chunks, normalize + j5
                # transposes (interleaved per chunk), then the output proj
                av_chunks7(6, av7)
                av_chunks7(7, av7)
                normalize7(av7)
                out_proj()

    nc.compile()
    return nc


def _get_nc(reps=1, skip_attention=False):
    key = f"nc{reps}_{skip_attention}"
    if key not in _CACHE:
        _CACHE[key] = _build_nc(reps, skip_attention)
    return _CACHE[key]


def _perm_cols(w):
    """[768, 768] -> [768, 1024]: head-pair column layout, zero padded."""
    out = np.zeros((D, 1024), np.float32)
    for p in range(PAIRS):
        out[:, 128 * p : 128 * p + 48] = w[:, 96 * p : 96 * p + 48]
        out[:, 128 * p + 64 : 128 * p + 112] = w[:, 96 * p + 48 : 96 * p + 96]
    return out


def _pack_bias_pairs(b):
    """[768] -> [128, 8]: per-pair per-partition bias columns."""
    t = np.zeros((128, PAIRS), np.float32)
    for p in range(PAIRS):
        t[0:48, p] = b[96 * p : 96 * p + 48]
        t[64:112, p] = b[96 * p + 48 : 96 * p + 96]
    return t


def _swizzle(w, nt, cols):
    """[nt*128, cols] -> [128, nt, cols] partition-major SBUF image."""
    return np.ascontiguousarray(w.reshape(nt, 128, cols).transpose(1, 0, 2))


def kernel(x, Wq, bq, Wk, bk, Wv, bv, Wo, bo, _trace=False):
    global LAST_RESULTS, LAST_IN_MAPS
    from concourse.bass_utils import run_bass_kernel_spmd

    x = np.asarray(x, np.float32)
    Wq = np.asarray(Wq, np.float32)
    Wk = np.asarray(Wk, np.float32)
    Wv = np.asarray(Wv, np.float32)
    Wo = np.asarray(Wo, np.float32)
    bq = np.asarray(bq, np.float32)
    bk = np.asarray(bk, np.float32)
    bv = np.asarray(bv, np.float32)
    bo = np.asarray(bo, np.float32)

    s = np.float32(1.0 / math.sqrt(HD))
    # [128, 6, 1024] -> pair-outermost [8, 128, 6, 128] so each pair is one
    # contiguous DMA (and pairs 1-7 a single bulk DMA)
    wq_p = np.ascontiguousarray(
        _swizzle(_perm_cols(Wq * s).astype(np.float16), 6, 1024)
        .reshape(128, 6, 8, 128)
        .transpose(2, 0, 1, 3)
    )
    wk_p = np.ascontiguousarray(
        _swizzle(_perm_cols(Wk).astype(np.float16), 6, 1024)
        .reshape(128, 6, 8, 128)
        .transpose(2, 0, 1, 3)
    )
    wv_p = _swizzle(Wv.astype(np.float16), 6, 768)
    wo_p = _swizzle(Wo.astype(np.float16), 6, 768)
    bb = np.zeros((128, 22), np.float32)
    bb[:, 0:8] = _pack_bias_pairs(bq * s)
    bb[:, 8:16] = _pack_bias_pairs(bk)
    bo_eff = bo + bv @ Wo
    bb[:, 16:22] = bo_eff.reshape(6, 128).T

    x16 = x.astype(np.float16)  # [B, S, D]

    shared = {"wq": wq_p, "wk": wk_p, "wv": wv_p, "wo": wo_p, "bb": bb}
    in_maps = [
        dict(shared, x=np.ascontiguousarray(x16[i].T).reshape(6, 128, 1024))
        for i in range(N_CORES)
    ]
    LAST_IN_MAPS = in_maps

    nc = _get_nc()
    try:
        res = run_bass_kernel_spmd(
            nc, in_maps, core_ids=list(range(N_CORES)), trace=_trace
        )
    except ModuleNotFoundError:
        # no axon NTFF profiling hook in this container
        res = run_bass_kernel_spmd(nc, in_maps, core_ids=list(range(N_CORES)))
    LAST_RESULTS = res

    y = np.stack([res.results[i]["y"].T for i in range(N_CORES)])  # [B, S, D]
    return np.ascontiguousarray(y.astype(np.float32))
